# revision 35
# baseline (speedup 1.0000x reference)
"""CALoraLinear kernel for 8 TRN2 NeuronCores (Bass/Tile, SPMD).

Math (derived from the reference):
  orig = x @ W.T + bias
  top2 classes c1,c2 per row from pseudo_index[b, :64]
  g_j = <lora_A[c_j], x[b]>          (only rows 0..63 of lora_A are reachable)
  lora_out[b,o] = 16 * sum_c mask[b,c] * G[b,c] * lora_B[o,c]
  out = orig + lora_out + bias       (bias added twice)

Sharding: column-shard W across the 8 cores (each core owns 512 output
columns, full batch); x / lora_A / pseudo_index replicated. Host
concatenates the per-core [512, 512] blocks along the output axis.
(An 8-core G k-split with a DRAM AllReduce was tried and rejected: the
collective measures ~18us internally and starts tens of us late in this
runtime, and enabling collectives adds a global barrier to the preamble.)

Schedule: fp16 operand stream (PE upconverts to FP22; ~3e-4 rel err,
half the DMA bytes of f32r). Four single-k-tile chunks lead (fast-
landing, so the PE builds an SBUF backlog that rides out DMA jitter
without gaps — a PE gap drops the clock p-state and costs ~2.5us),
then 14 double-k-tile chunks. Chunk DMAs alternate the two HWDGE rings
with a window-2 gate per ring: each trigger waits for the
before-previous transfer on its ring, since with more in flight the
DMA queues round-robin across every outstanding transfer and the
first chunk's completion slips by many us. pp (pseudo_index + lora_B
block) rides the GPSIMD SWDGE ring mid-stream. Six dummy matmuls on a
zeroed tile warm the PE clock out of its cold p-state during the
first-chunk DMA wait (measured: steady-state is 216ns per
128x128x512 matmul, cold is 427-610ns, and full clock needs ~3us of
continuous PE activity).

The per-k-tile G matmul uses a 128-col stationary window ending at the
a-block (64 garbage w-cols + the 64 real a-cols, real G in PSUM rows
64:128): a 64-wide stationary forces a PE pipeline-drain costing
~110ns on the G matmul AND the following main (measured 333/322 vs
216ns). The last chunk runs both G matmuls before its mains so the
DVE ht chain overlaps them; ht/bS are padded to a 128-row contraction
for the same geometry reason. The tail matmuls close each PSUM bank
with stop=True; per-bank f16 copy-out (half the bytes, host upcasts)
and store DMAs on three different engines pipeline the epilogue.

Evaluated and rejected:
- fp8 e4m3 mains: measures 3.4e-2 full-output rel err, over the 2e-2
  gate (DoubleRow's 2x PE rate can't pay for the 3-matmul error
  compensation it would need).
- G k-split across cores + DRAM AllReduce (would remove 28 of 32
  replicated G matmuls): the collective measures ~18us internally and
  starts tens of us late in this runtime, and enabling collectives
  adds a multi-us global barrier to the preamble.
"""

import os
import sys

for _p in ("/opt/trn_rl_repo",):
    if _p not in sys.path:
        sys.path.insert(0, _p)

import numpy as np

import concourse.bass as bass
import concourse.bass_isa as bass_isa
import concourse.bacc as bacc
import concourse.mybir as mybir
from concourse.tile import TileContext, add_dep_helper
from concourse.bass_utils import run_bass_kernel_spmd


def _ensure_ntff_hook_module():
    """run_bass_kernel_spmd(trace=True) imports antenv.axon_hooks, which the
    agent image's antenv package lacks. Provide it (and register the real
    ctypes NTFF hook when available) so a tracing caller doesn't crash."""
    import types

    try:
        import antenv
    except ImportError:
        return
    if getattr(antenv, "axon_hooks", None) is not None:
        return
    mod = types.ModuleType("antenv.axon_hooks")
    state = {"hook": None}
    mod.set_axon_ntff_profile_hook = lambda h: state.__setitem__("hook", h)
    mod.get_axon_ntff_profile_hook = lambda: state["hook"]
    sys.modules["antenv.axon_hooks"] = mod
    antenv.axon_hooks = mod
    try:
        from trn_agent_boot.trn_boot import _ntff_profile_via_ctypes

        mod.set_axon_ntff_profile_hook(
            _ntff_profile_via_ctypes("/opt/axon/libaxon_pjrt.so")
        )
    except Exception:
        pass


_ensure_ntff_hook_module()

B, IN, OUT = 512, 4096, 4096
NUM_CLASS, RANK = 64, 8
NCORES = 8
OUT_L = OUT // NCORES  # 512
P = 128
KT = IN // P           # 32 k-tiles
BT = B // P            # 4 batch tiles

NSING = 6                  # leading single-k-tile chunks (PE-side cushion:
                           # small chunks land fast, so the stream builds an
                           # SBUF backlog that rides out DMA jitter without
                           # PE gaps, which would drop the clock p-state)
NDBL = (KT - NSING) // 2   # 13 double-k-tile chunks

# single chunk columns: [x: B][w: OUT_L][a: 64]
S_XOFF, S_WOFF, S_AOFF = 0, B, B + OUT_L
SW = B + OUT_L + NUM_CLASS                 # 1088
# double chunk columns: [x0][x1][w0][w1][a0][a1]
D_XOFF, D_WOFF, D_AOFF = 0, 2 * B, 2 * (B + OUT_L)
DW = 2 * SW                                # 2176

# pp layout: [psT: B][bS: OUT_L (rows 0:65, rest zero)]
PTOFF = 0
BSOFF = PTOFF + B
PPW = BSOFF + OUT_L

F32 = mybir.dt.float32
F32R = mybir.dt.float32r
F16 = mybir.dt.float16
X = mybir.AxisListType.X

NWARM = int(os.environ.get("NWARM", "6"))

_cache = {}
# test.py reads this after a traced run for HW exec time
last_results = None


def _build():
    key = f"nc_w{NWARM}"
    if key in _cache:
        return _cache[key]
    nc = bacc.Bacc(
        bass.get_trn_type() or "TRN2",
        target_bir_lowering=False,
        debug=False,
        num_devices=NCORES,
    )

    xw_s = nc.dram_tensor("xw_s", [NSING, P, SW], F16, kind="ExternalInput")
    xw_d = nc.dram_tensor("xw_d", [NDBL, P, DW], F16, kind="ExternalInput")
    pp = nc.dram_tensor("pp", [P, PPW], F32R, kind="ExternalInput")
    # f16 output staging: halves the PSUM->SBUF copy and store-DMA bytes on
    # the critical tail; the host upcasts. Adds ~2^-12 RMS rounding on top
    # of the fp16 stream's ~3e-4 rel err (measured total 3.9e-4).
    out = nc.dram_tensor("out", [B, OUT_L], F16, kind="ExternalOutput")

    with TileContext(nc) as tc:
        with (
            tc.tile_pool(name="xwp", bufs=1) as xwpool,
            tc.tile_pool(name="sml", bufs=1) as spool,
            tc.tile_pool(name="tl", bufs=1) as tpool,
            tc.tile_pool(name="op", bufs=1) as opool,
            tc.tile_pool(name="dr", bufs=1, space="DRAM") as dpool,
            tc.tile_pool(name="ps", bufs=1, space="PSUM") as ppool,
        ):
            # ---- PE warmup: dummy matmuls ramp the PE clock out of its
            # cold p-state while the first chunk DMA is in flight. The
            # result bank is never read.
            if NWARM:
                wt = spool.tile([P, P + OUT_L], F16, tag="warm")
                nc.vector.memset(wt, 0.0)
                warm_ps = ppool.tile([P, OUT_L], F32, tag="warm", name="warm")
                for _ in range(NWARM):
                    nc.tensor.matmul(
                        warm_ps,
                        lhsT=wt[:, :P],
                        rhs=wt[:, P : P + OUT_L],
                        start=True,
                        stop=True,
                    )

            # ---- input DMA triggers on the two HWDGE rings, window-2 gated:
            # each ring holds at most 2 in-flight transfers. With more, the
            # DMA queues round-robin across every outstanding transfer and
            # the FIRST chunk's completion slips by many us (measured: first
            # matmul at 18.6us instead of ~10us when all 17 were issued
            # up-front). pp is sequenced mid-stream on the scalar ring: its
            # consumers (top-2 mask, bS) aren't needed until stream end.
            s_tiles = [
                xwpool.tile([P, SW], F16, tag=f"s{c}", name=f"s{c}")
                for c in range(NSING)
            ]
            d_tiles = [
                xwpool.tile([P, DW], F16, tag=f"d{c}", name=f"d{c}")
                for c in range(NDBL)
            ]
            pp_sb = spool.tile([P, PPW], F32R)
            prev_dma = {0: None, 1: None}  # per-ring (c-2) gating chain
            gate_dma = {0: None, 1: None}

            def issue(eng_i, out_tile, src):
                eng = nc.sync if eng_i == 0 else nc.scalar
                dma = eng.dma_start(out=out_tile, in_=src)
                if gate_dma[eng_i] is not None:
                    add_dep_helper(
                        dma.ins,
                        gate_dma[eng_i].ins,
                        reason="window-2 DMA gating per ring",
                    )
                gate_dma[eng_i] = prev_dma[eng_i]
                prev_dma[eng_i] = dma
                return dma

            for s in range(NSING):
                issue(s % 2, s_tiles[s], xw_s[s])
            d_dmas = []
            for c in range(NDBL):
                d_dmas.append(issue(c % 2, d_tiles[c], xw_d[c]))
            # pp rides the GPSIMD SWDGE ring so it never crowds the chunk
            # stream; gated until d1 lands (its consumers run mid-stream)
            pp_dma = nc.gpsimd.dma_start(out=pp_sb, in_=pp[:, :])
            add_dep_helper(
                pp_dma.ins, d_dmas[1].ins, reason="keep pp off the early chunks"
            )

            psT_sb = pp_sb[:NUM_CLASS, PTOFF : PTOFF + B].bitcast(F32)
            bS_sb = pp_sb[:, BSOFF : BSOFF + OUT_L]  # rows 65:128 are zeros

            # ---- PSUM accumulators ----
            mps = [
                ppool.tile([P, OUT_L], F32, tag=f"main{bt}", name=f"main{bt}")
                for bt in range(BT)
            ]
            # G accumulator: full 128-partition bank; the real G lives in
            # rows 64:128. The G matmul's stationary is a 128-col window
            # ending at the a-block (64 w-cols of garbage + the 64 a-cols),
            # so its geometry matches the main matmuls exactly — a 64-wide
            # stationary forces a PE pipeline drain (~110ns) on the G matmul
            # AND on the following main (measured 333/322 vs 216ns).
            gt_ps = ppool.tile([P, B], F32, tag="gt", name="gt_ps")

            def do_k(xk, wk, a128, kidx):
                # G first: at the last k-tile this lets the DVE ht chain
                # overlap the final main matmuls
                nc.tensor.matmul(
                    gt_ps,
                    lhsT=a128,
                    rhs=xk,
                    start=(kidx == 0),
                    stop=(kidx == KT - 1),
                )
                for bt in range(BT):
                    nc.tensor.matmul(
                        mps[bt],
                        lhsT=xk[:, bt * P : (bt + 1) * P],
                        rhs=wk,
                        start=(kidx == 0),
                        stop=False,
                    )

            kidx = 0
            for s in range(NSING):
                t = s_tiles[s]
                do_k(
                    t[:, S_XOFF : S_XOFF + B],
                    t[:, S_WOFF : S_WOFF + OUT_L],
                    t[:, S_AOFF - NUM_CLASS : S_AOFF + NUM_CLASS],
                    kidx,
                )
                kidx += 1

            # ---- top-2 threshold + mask, computed directly in the
            # [class-partition, batch-free] orientation with GPSIMD
            # cross-partition all-reduces. (The previous partition->free
            # DRAM bounce rode the SWDGE path, which starves while the
            # HWDGE chunk stream runs — measured landing ~15us late and
            # stalling the tail matmuls.)
            psT2 = tpool.tile([NUM_CLASS, B], F32)
            nc.vector.tensor_copy(out=psT2, in_=psT_sb)
            m1a = tpool.tile([NUM_CLASS, B], F32)
            nc.gpsimd.partition_all_reduce(
                m1a, psT2, NUM_CLASS, bass_isa.ReduceOp.max
            )
            # kill the per-column max, then the next all-reduce max is the
            # 2nd max = top-2 threshold (replicated across partitions)
            negm = tpool.tile([NUM_CLASS, B], F32)
            nc.vector.tensor_tensor(
                out=negm, in0=psT2, in1=m1a, op=mybir.AluOpType.is_ge
            )
            nc.vector.tensor_scalar(
                out=negm,
                in0=negm,
                scalar1=-1.0e30,
                scalar2=None,
                op0=mybir.AluOpType.mult,
            )
            nc.vector.tensor_tensor(
                out=negm, in0=psT2, in1=negm, op=mybir.AluOpType.add
            )
            thr = tpool.tile([NUM_CLASS, B], F32)
            nc.gpsimd.partition_all_reduce(
                thr, negm, NUM_CLASS, bass_isa.ReduceOp.max
            )
            maskT = tpool.tile([NUM_CLASS, B], F32)
            nc.vector.tensor_tensor(
                out=maskT, in0=psT2, in1=thr, op=mybir.AluOpType.is_ge
            )
            # ht padded to a full 128-row contraction (rows 65:128 zero, with
            # matching zero rows in bS) so the tail matmul keeps the same
            # 128-partition geometry as the stream and avoids the PE drain.
            ht = tpool.tile([P, B], F32R)
            # zero rows 64:128 (aligned base), then the ones row at 64
            # (Memset can't write float32r, hence the in0*0+c idiom)
            nc.vector.tensor_scalar(
                out=ht[NUM_CLASS:P, :],
                in0=psT2[0 : P - NUM_CLASS, :],
                scalar1=0.0,
                scalar2=0.0,
                op0=mybir.AluOpType.mult,
                op1=mybir.AluOpType.add,
            )
            nc.vector.tensor_scalar(
                out=ht[NUM_CLASS : NUM_CLASS + 1, :],
                in0=psT2[0:1, :],
                scalar1=0.0,
                scalar2=1.0,
                op0=mybir.AluOpType.mult,
                op1=mybir.AluOpType.add,
            )

            # ---- double chunks ----
            def d_parts(c, kk):
                t = d_tiles[c]
                return (
                    t[:, D_XOFF + kk * B : D_XOFF + (kk + 1) * B],
                    t[:, D_WOFF + kk * OUT_L : D_WOFF + (kk + 1) * OUT_L],
                    t[
                        :,
                        D_AOFF
                        + (kk - 1) * NUM_CLASS : D_AOFF
                        + (kk + 1) * NUM_CLASS,
                    ],
                )

            for c in range(NDBL - 1):
                for kk in range(2):
                    xk, wk, a128 = d_parts(c, kk)
                    do_k(xk, wk, a128, kidx)
                    kidx += 1

            # last chunk: both G matmuls first so the ht chain overlaps the
            # final mains; then banks close EARLY->LATE (mains kk0/kk1 + the
            # rank-128 LoRA tail per bank, no two consecutive instructions on
            # one bank), so each bank's copy-out + store DMA pipeline against
            # the remaining matmuls.
            last = NDBL - 1
            for kk in range(2):
                xk, _, a128 = d_parts(last, kk)
                nc.tensor.matmul(
                    gt_ps,
                    lhsT=a128,
                    rhs=xk,
                    start=False,
                    stop=(kk == 1),
                )
            kidx += 2

            # ht = G * mask in per-bank column blocks (subtile deps let each
            # tail matmul start as soon as its block is ready), reading the G
            # accumulator PSUM directly (real G in rows 64:128)
            for bt in range(BT):
                nc.vector.tensor_tensor(
                    out=ht[0:NUM_CLASS, bt * P : (bt + 1) * P],
                    in0=gt_ps[NUM_CLASS : 2 * NUM_CLASS, bt * P : (bt + 1) * P],
                    in1=maskT[:, bt * P : (bt + 1) * P],
                    op=mybir.AluOpType.mult,
                )

            o_all = opool.tile([P, BT * OUT_L], F16)

            def close_bank(bt):
                nc.tensor.matmul(
                    mps[bt],
                    lhsT=ht[:, bt * P : (bt + 1) * P],
                    rhs=bS_sb,
                    start=False,
                    stop=True,
                )
                dst = o_all[:, bt * OUT_L : (bt + 1) * OUT_L]
                # copies alternate vector/scalar; store triggers spread
                # across three engines, so per-bank copy+issue overlap
                if bt % 2 == 0:
                    nc.vector.tensor_copy(out=dst, in_=mps[bt])
                else:
                    nc.scalar.copy(out=dst, in_=mps[bt])
                eng = (nc.sync, nc.gpsimd, nc.sync, nc.scalar)[bt]
                eng.dma_start(out=out[bt * P : (bt + 1) * P, :], in_=dst)

            def m_last(kk, bt):
                xk, wk, _ = d_parts(last, kk)
                nc.tensor.matmul(
                    mps[bt],
                    lhsT=xk[:, bt * P : (bt + 1) * P],
                    rhs=wk,
                    start=False,
                    stop=False,
                )

            for b0 in (0, 2):
                m_last(0, b0)
                m_last(0, b0 + 1)
                m_last(1, b0)
                m_last(1, b0 + 1)
                close_bank(b0)
                close_bank(b0 + 1)

    nc.finalize()
    _cache[key] = nc
    return nc


def _pack_inputs(x, pseudo_index, weight, bias, lora_A, lora_B):
    """Build the per-core chunked xw buffers + replicated small inputs."""
    xT = np.ascontiguousarray(x.T).astype(np.float16)        # [IN, B]
    aT = np.ascontiguousarray(
        lora_A[:NUM_CLASS].T
    ).astype(np.float16)                                     # [IN, 64]

    pp_base = np.zeros((P, PPW), dtype=np.float32)
    pp_base[:NUM_CLASS, PTOFF : PTOFF + B] = pseudo_index.T

    in_maps = []
    for i in range(NCORES):
        o0 = i * OUT_L
        wTi = weight[o0 : o0 + OUT_L].T.astype(np.float16)   # [IN, OUT_L]

        xws = np.empty((NSING, P, SW), dtype=np.float16)
        for s in range(NSING):
            xws[s, :, S_XOFF : S_XOFF + B] = xT[s * P : (s + 1) * P]
            xws[s, :, S_WOFF : S_WOFF + OUT_L] = wTi[s * P : (s + 1) * P]
            xws[s, :, S_AOFF : S_AOFF + NUM_CLASS] = aT[s * P : (s + 1) * P]

        xwd = np.empty((NDBL, P, DW), dtype=np.float16)
        for c in range(NDBL):
            k0 = NSING + 2 * c
            k1 = k0 + 1
            xwd[c, :, D_XOFF : D_XOFF + B] = xT[k0 * P : (k0 + 1) * P]
            xwd[c, :, D_XOFF + B : D_XOFF + 2 * B] = xT[k1 * P : (k1 + 1) * P]
            xwd[c, :, D_WOFF : D_WOFF + OUT_L] = wTi[k0 * P : (k0 + 1) * P]
            xwd[c, :, D_WOFF + OUT_L : D_WOFF + 2 * OUT_L] = wTi[
                k1 * P : (k1 + 1) * P
            ]
            xwd[c, :, D_AOFF : D_AOFF + NUM_CLASS] = aT[k0 * P : (k0 + 1) * P]
            xwd[c, :, D_AOFF + NUM_CLASS : DW] = aT[k1 * P : (k1 + 1) * P]

        ppi = pp_base.copy()
        ppi[:NUM_CLASS, BSOFF : BSOFF + OUT_L] = (
            16.0 * lora_B[o0 : o0 + OUT_L, :NUM_CLASS].T
        )
        ppi[NUM_CLASS, BSOFF : BSOFF + OUT_L] = 2.0 * bias[o0 : o0 + OUT_L]
        in_maps.append({"xw_s": xws, "xw_d": xwd, "pp": ppi})
    return in_maps


def kernel(x, pseudo_index, weight, bias, lora_A, lora_B):
    global last_results
    x = np.ascontiguousarray(np.asarray(x, dtype=np.float32))
    pseudo_index = np.ascontiguousarray(np.asarray(pseudo_index, dtype=np.float32))
    weight = np.asarray(weight, dtype=np.float32)
    bias = np.asarray(bias, dtype=np.float32)
    lora_A = np.asarray(lora_A, dtype=np.float32)
    lora_B = np.asarray(lora_B, dtype=np.float32)

    nc = _build()
    in_maps = _pack_inputs(x, pseudo_index, weight, bias, lora_A, lora_B)
    res = run_bass_kernel_spmd(nc, in_maps, list(range(NCORES)))
    last_results = res
    return np.hstack(
        [res.results[i]["out"].astype(np.float32) for i in range(NCORES)]
    )


# revision 36
# speedup vs baseline: 1.1177x; 1.1177x over previous
"""CALoraLinear kernel for 8 TRN2 NeuronCores (Bass/Tile, SPMD).

Math (derived from the reference):
  orig = x @ W.T + bias
  top2 classes c1,c2 per row from pseudo_index[b, :64]
  g_j = <lora_A[c_j], x[b]>          (only rows 0..63 of lora_A are reachable)
  lora_out[b,o] = 16 * sum_c mask[b,c] * G[b,c] * lora_B[o,c]
  out = orig + lora_out + bias       (bias added twice)

Sharding: column-shard W across the 8 cores (each core owns 512 output
columns, full batch); x / lora_A / pseudo_index replicated. Host
concatenates the per-core [512, 512] blocks along the output axis.
(An 8-core G k-split with a DRAM AllReduce was tried and rejected: the
collective measures ~18us internally and starts tens of us late in this
runtime, and enabling collectives adds a global barrier to the preamble.)

Schedule: fp16 operand stream (PE upconverts to FP22; ~3e-4 rel err,
half the DMA bytes of f32r). Four single-k-tile chunks lead (fast-
landing, so the PE builds an SBUF backlog that rides out DMA jitter
without gaps — a PE gap drops the clock p-state and costs ~2.5us),
then 14 double-k-tile chunks. Chunk DMAs alternate the two HWDGE rings
with a window-2 gate per ring: each trigger waits for the
before-previous transfer on its ring, since with more in flight the
DMA queues round-robin across every outstanding transfer and the
first chunk's completion slips by many us. pp (pseudo_index + lora_B
block) rides the GPSIMD SWDGE ring mid-stream. Six dummy matmuls on a
zeroed tile warm the PE clock out of its cold p-state during the
first-chunk DMA wait (measured: steady-state is 216ns per
128x128x512 matmul, cold is 427-610ns, and full clock needs ~3us of
continuous PE activity).

The per-k-tile G matmul uses a 128-col stationary window ending at the
a-block (64 garbage w-cols + the 64 real a-cols, real G in PSUM rows
64:128): a 64-wide stationary forces a PE pipeline-drain costing
~110ns on the G matmul AND the following main (measured 333/322 vs
216ns). The top-2 threshold/mask is computed entirely in the
[class-partition, batch-free] orientation with two GPSIMD
partition_all_reduce maxes (a DRAM-bounce transpose on the SWDGE path
starves behind the HWDGE chunk stream and landed ~15us late). The
last chunk runs both G matmuls first, then closes the four PSUM banks
early->late (mains + rank-128 LoRA tail per bank) so each bank's f16
copy-out (vector/scalar alternating, host upcasts) and store DMA
(triggers on three engines) pipeline against the remaining matmuls;
ht/bS are padded to a 128-row contraction to keep tail geometry
uniform, and ht is produced in per-bank column blocks so each tail
starts as soon as its block is ready.

Evaluated and rejected:
- fp8 e4m3 mains: measures 3.4e-2 full-output rel err, over the 2e-2
  gate (DoubleRow's 2x PE rate can't pay for the 3-matmul error
  compensation it would need).
- G k-split across cores + DRAM AllReduce (would remove 28 of 32
  replicated G matmuls): the collective measures ~18us internally and
  starts tens of us late in this runtime, and enabling collectives
  adds a multi-us global barrier to the preamble.
"""

import os
import sys

for _p in ("/opt/trn_rl_repo",):
    if _p not in sys.path:
        sys.path.insert(0, _p)

import numpy as np

import concourse.bass as bass
import concourse.bass_isa as bass_isa
import concourse.bacc as bacc
import concourse.mybir as mybir
from concourse.tile import TileContext, add_dep_helper
from concourse.bass_utils import run_bass_kernel_spmd


def _ensure_ntff_hook_module():
    """run_bass_kernel_spmd(trace=True) imports antenv.axon_hooks, which the
    agent image's antenv package lacks. Provide it (and register the real
    ctypes NTFF hook when available) so a tracing caller doesn't crash."""
    import types

    try:
        import antenv
    except ImportError:
        return
    if getattr(antenv, "axon_hooks", None) is not None:
        return
    mod = types.ModuleType("antenv.axon_hooks")
    state = {"hook": None}
    mod.set_axon_ntff_profile_hook = lambda h: state.__setitem__("hook", h)
    mod.get_axon_ntff_profile_hook = lambda: state["hook"]
    sys.modules["antenv.axon_hooks"] = mod
    antenv.axon_hooks = mod
    try:
        from trn_agent_boot.trn_boot import _ntff_profile_via_ctypes

        mod.set_axon_ntff_profile_hook(
            _ntff_profile_via_ctypes("/opt/axon/libaxon_pjrt.so")
        )
    except Exception:
        pass


_ensure_ntff_hook_module()

B, IN, OUT = 512, 4096, 4096
NUM_CLASS, RANK = 64, 8
NCORES = 8
OUT_L = OUT // NCORES  # 512
P = 128
KT = IN // P           # 32 k-tiles
BT = B // P            # 4 batch tiles

NSING = 6                  # leading single-k-tile chunks (PE-side cushion:
                           # small chunks land fast, so the stream builds an
                           # SBUF backlog that rides out DMA jitter without
                           # PE gaps, which would drop the clock p-state)
NDBL = (KT - NSING) // 2   # 13 double-k-tile chunks

# single chunk columns: [x: B][w: OUT_L][a: 64]
S_XOFF, S_WOFF, S_AOFF = 0, B, B + OUT_L
SW = B + OUT_L + NUM_CLASS                 # 1088
# double chunk columns: [x0][x1][w0][w1][a0][a1]
D_XOFF, D_WOFF, D_AOFF = 0, 2 * B, 2 * (B + OUT_L)
DW = 2 * SW                                # 2176

# pp layout: [psT: B][bS: OUT_L (rows 0:65, rest zero)]
PTOFF = 0
BSOFF = PTOFF + B
PPW = BSOFF + OUT_L

F32 = mybir.dt.float32
F32R = mybir.dt.float32r
F16 = mybir.dt.float16
X = mybir.AxisListType.X

NWARM = int(os.environ.get("NWARM", "6"))

_cache = {}
# test.py reads this after a traced run for HW exec time
last_results = None


def _build():
    key = f"nc_w{NWARM}"
    if key in _cache:
        return _cache[key]
    nc = bacc.Bacc(
        bass.get_trn_type() or "TRN2",
        target_bir_lowering=False,
        debug=False,
        num_devices=NCORES,
    )

    xw_s = nc.dram_tensor("xw_s", [NSING, P, SW], F16, kind="ExternalInput")
    xw_d = nc.dram_tensor("xw_d", [NDBL, P, DW], F16, kind="ExternalInput")
    pp = nc.dram_tensor("pp", [P, PPW], F32R, kind="ExternalInput")
    # f16 output staging: halves the PSUM->SBUF copy and store-DMA bytes on
    # the critical tail; the host upcasts. Adds ~2^-12 RMS rounding on top
    # of the fp16 stream's ~3e-4 rel err (measured total 3.9e-4).
    out = nc.dram_tensor("out", [B, OUT_L], F16, kind="ExternalOutput")

    with TileContext(nc) as tc:
        with (
            tc.tile_pool(name="xwp", bufs=1) as xwpool,
            tc.tile_pool(name="sml", bufs=1) as spool,
            tc.tile_pool(name="tl", bufs=1) as tpool,
            tc.tile_pool(name="op", bufs=1) as opool,
            tc.tile_pool(name="dr", bufs=1, space="DRAM") as dpool,
            tc.tile_pool(name="ps", bufs=1, space="PSUM") as ppool,
        ):
            # ---- PE warmup: dummy matmuls ramp the PE clock out of its
            # cold p-state while the first chunk DMA is in flight. The
            # result bank is never read.
            if NWARM:
                wt = spool.tile([P, P + OUT_L], F16, tag="warm")
                nc.vector.memset(wt, 0.0)
                warm_ps = ppool.tile([P, OUT_L], F32, tag="warm", name="warm")
                for _ in range(NWARM):
                    nc.tensor.matmul(
                        warm_ps,
                        lhsT=wt[:, :P],
                        rhs=wt[:, P : P + OUT_L],
                        start=True,
                        stop=True,
                    )

            # ---- input DMA triggers on the two HWDGE rings, window-2 gated:
            # each ring holds at most 2 in-flight transfers. With more, the
            # DMA queues round-robin across every outstanding transfer and
            # the FIRST chunk's completion slips by many us (measured: first
            # matmul at 18.6us instead of ~10us when all 17 were issued
            # up-front). pp is sequenced mid-stream on the scalar ring: its
            # consumers (top-2 mask, bS) aren't needed until stream end.
            s_tiles = [
                xwpool.tile([P, SW], F16, tag=f"s{c}", name=f"s{c}")
                for c in range(NSING)
            ]
            d_tiles = [
                xwpool.tile([P, DW], F16, tag=f"d{c}", name=f"d{c}")
                for c in range(NDBL)
            ]
            pp_sb = spool.tile([P, PPW], F32R)
            prev_dma = {0: None, 1: None}  # per-ring (c-2) gating chain
            gate_dma = {0: None, 1: None}

            def issue(eng_i, out_tile, src):
                eng = nc.sync if eng_i == 0 else nc.scalar
                dma = eng.dma_start(out=out_tile, in_=src)
                if gate_dma[eng_i] is not None:
                    add_dep_helper(
                        dma.ins,
                        gate_dma[eng_i].ins,
                        reason="window-2 DMA gating per ring",
                    )
                gate_dma[eng_i] = prev_dma[eng_i]
                prev_dma[eng_i] = dma
                return dma

            for s in range(NSING):
                issue(s % 2, s_tiles[s], xw_s[s])
            d_dmas = []
            for c in range(NDBL):
                d_dmas.append(issue(c % 2, d_tiles[c], xw_d[c]))
            # pp rides the GPSIMD SWDGE ring so it never crowds the chunk
            # stream; gated until d1 lands (its consumers run mid-stream)
            pp_dma = nc.gpsimd.dma_start(out=pp_sb, in_=pp[:, :])
            add_dep_helper(
                pp_dma.ins, d_dmas[1].ins, reason="keep pp off the early chunks"
            )

            psT_sb = pp_sb[:NUM_CLASS, PTOFF : PTOFF + B].bitcast(F32)
            bS_sb = pp_sb[:, BSOFF : BSOFF + OUT_L]  # rows 65:128 are zeros

            # ---- PSUM accumulators ----
            mps = [
                ppool.tile([P, OUT_L], F32, tag=f"main{bt}", name=f"main{bt}")
                for bt in range(BT)
            ]
            # G accumulator: full 128-partition bank; the real G lives in
            # rows 64:128. The G matmul's stationary is a 128-col window
            # ending at the a-block (64 w-cols of garbage + the 64 a-cols),
            # so its geometry matches the main matmuls exactly — a 64-wide
            # stationary forces a PE pipeline drain (~110ns) on the G matmul
            # AND on the following main (measured 333/322 vs 216ns).
            gt_ps = ppool.tile([P, B], F32, tag="gt", name="gt_ps")

            def do_k(xk, wk, a128, kidx):
                # G first: at the last k-tile this lets the DVE ht chain
                # overlap the final main matmuls
                nc.tensor.matmul(
                    gt_ps,
                    lhsT=a128,
                    rhs=xk,
                    start=(kidx == 0),
                    stop=(kidx == KT - 1),
                )
                for bt in range(BT):
                    nc.tensor.matmul(
                        mps[bt],
                        lhsT=xk[:, bt * P : (bt + 1) * P],
                        rhs=wk,
                        start=(kidx == 0),
                        stop=False,
                    )

            kidx = 0
            for s in range(NSING):
                t = s_tiles[s]
                do_k(
                    t[:, S_XOFF : S_XOFF + B],
                    t[:, S_WOFF : S_WOFF + OUT_L],
                    t[:, S_AOFF - NUM_CLASS : S_AOFF + NUM_CLASS],
                    kidx,
                )
                kidx += 1

            # ---- top-2 threshold + mask, computed directly in the
            # [class-partition, batch-free] orientation with GPSIMD
            # cross-partition all-reduces. (The previous partition->free
            # DRAM bounce rode the SWDGE path, which starves while the
            # HWDGE chunk stream runs — measured landing ~15us late and
            # stalling the tail matmuls.)
            psT2 = tpool.tile([NUM_CLASS, B], F32)
            nc.vector.tensor_copy(out=psT2, in_=psT_sb)
            m1a = tpool.tile([NUM_CLASS, B], F32)
            nc.gpsimd.partition_all_reduce(
                m1a, psT2, NUM_CLASS, bass_isa.ReduceOp.max
            )
            # kill the per-column max, then the next all-reduce max is the
            # 2nd max = top-2 threshold (replicated across partitions)
            negm = tpool.tile([NUM_CLASS, B], F32)
            nc.vector.tensor_tensor(
                out=negm, in0=psT2, in1=m1a, op=mybir.AluOpType.is_ge
            )
            nc.vector.tensor_scalar(
                out=negm,
                in0=negm,
                scalar1=-1.0e30,
                scalar2=None,
                op0=mybir.AluOpType.mult,
            )
            nc.vector.tensor_tensor(
                out=negm, in0=psT2, in1=negm, op=mybir.AluOpType.add
            )
            thr = tpool.tile([NUM_CLASS, B], F32)
            nc.gpsimd.partition_all_reduce(
                thr, negm, NUM_CLASS, bass_isa.ReduceOp.max
            )
            maskT = tpool.tile([NUM_CLASS, B], F32)
            nc.vector.tensor_tensor(
                out=maskT, in0=psT2, in1=thr, op=mybir.AluOpType.is_ge
            )
            # ht padded to a full 128-row contraction (rows 65:128 zero, with
            # matching zero rows in bS) so the tail matmul keeps the same
            # 128-partition geometry as the stream and avoids the PE drain.
            ht = tpool.tile([P, B], F32R)
            # zero rows 64:128 (aligned base), then the ones row at 64
            # (Memset can't write float32r, hence the in0*0+c idiom)
            nc.vector.tensor_scalar(
                out=ht[NUM_CLASS:P, :],
                in0=psT2[0 : P - NUM_CLASS, :],
                scalar1=0.0,
                scalar2=0.0,
                op0=mybir.AluOpType.mult,
                op1=mybir.AluOpType.add,
            )
            nc.vector.tensor_scalar(
                out=ht[NUM_CLASS : NUM_CLASS + 1, :],
                in0=psT2[0:1, :],
                scalar1=0.0,
                scalar2=1.0,
                op0=mybir.AluOpType.mult,
                op1=mybir.AluOpType.add,
            )

            # ---- double chunks ----
            def d_parts(c, kk):
                t = d_tiles[c]
                return (
                    t[:, D_XOFF + kk * B : D_XOFF + (kk + 1) * B],
                    t[:, D_WOFF + kk * OUT_L : D_WOFF + (kk + 1) * OUT_L],
                    t[
                        :,
                        D_AOFF
                        + (kk - 1) * NUM_CLASS : D_AOFF
                        + (kk + 1) * NUM_CLASS,
                    ],
                )

            for c in range(NDBL - 1):
                for kk in range(2):
                    xk, wk, a128 = d_parts(c, kk)
                    do_k(xk, wk, a128, kidx)
                    kidx += 1

            # last chunk: both G matmuls first so the ht chain overlaps the
            # final mains; then banks close EARLY->LATE (mains kk0/kk1 + the
            # rank-128 LoRA tail per bank, no two consecutive instructions on
            # one bank), so each bank's copy-out + store DMA pipeline against
            # the remaining matmuls.
            last = NDBL - 1
            for kk in range(2):
                xk, _, a128 = d_parts(last, kk)
                nc.tensor.matmul(
                    gt_ps,
                    lhsT=a128,
                    rhs=xk,
                    start=False,
                    stop=(kk == 1),
                )
            kidx += 2

            # ht = G * mask in per-bank column blocks (subtile deps let each
            # tail matmul start as soon as its block is ready), reading the G
            # accumulator PSUM directly (real G in rows 64:128)
            for bt in range(BT):
                nc.vector.tensor_tensor(
                    out=ht[0:NUM_CLASS, bt * P : (bt + 1) * P],
                    in0=gt_ps[NUM_CLASS : 2 * NUM_CLASS, bt * P : (bt + 1) * P],
                    in1=maskT[:, bt * P : (bt + 1) * P],
                    op=mybir.AluOpType.mult,
                )

            o_all = opool.tile([P, BT * OUT_L], F16)

            def close_bank(bt):
                nc.tensor.matmul(
                    mps[bt],
                    lhsT=ht[:, bt * P : (bt + 1) * P],
                    rhs=bS_sb,
                    start=False,
                    stop=True,
                )
                dst = o_all[:, bt * OUT_L : (bt + 1) * OUT_L]
                # copies alternate vector/scalar; store triggers spread
                # across three engines, so per-bank copy+issue overlap
                if bt % 2 == 0:
                    nc.vector.tensor_copy(out=dst, in_=mps[bt])
                else:
                    nc.scalar.copy(out=dst, in_=mps[bt])
                eng = (nc.sync, nc.gpsimd, nc.sync, nc.scalar)[bt]
                eng.dma_start(out=out[bt * P : (bt + 1) * P, :], in_=dst)

            def m_last(kk, bt):
                xk, wk, _ = d_parts(last, kk)
                nc.tensor.matmul(
                    mps[bt],
                    lhsT=xk[:, bt * P : (bt + 1) * P],
                    rhs=wk,
                    start=False,
                    stop=False,
                )

            for b0 in (0, 2):
                m_last(0, b0)
                m_last(0, b0 + 1)
                m_last(1, b0)
                m_last(1, b0 + 1)
                close_bank(b0)
                close_bank(b0 + 1)

    nc.finalize()
    _cache[key] = nc
    return nc


def _pack_inputs(x, pseudo_index, weight, bias, lora_A, lora_B):
    """Build the per-core chunked xw buffers + replicated small inputs."""
    xT = np.ascontiguousarray(x.T).astype(np.float16)        # [IN, B]
    aT = np.ascontiguousarray(
        lora_A[:NUM_CLASS].T
    ).astype(np.float16)                                     # [IN, 64]

    pp_base = np.zeros((P, PPW), dtype=np.float32)
    pp_base[:NUM_CLASS, PTOFF : PTOFF + B] = pseudo_index.T

    in_maps = []
    for i in range(NCORES):
        o0 = i * OUT_L
        wTi = weight[o0 : o0 + OUT_L].T.astype(np.float16)   # [IN, OUT_L]

        xws = np.empty((NSING, P, SW), dtype=np.float16)
        for s in range(NSING):
            xws[s, :, S_XOFF : S_XOFF + B] = xT[s * P : (s + 1) * P]
            xws[s, :, S_WOFF : S_WOFF + OUT_L] = wTi[s * P : (s + 1) * P]
            xws[s, :, S_AOFF : S_AOFF + NUM_CLASS] = aT[s * P : (s + 1) * P]

        xwd = np.empty((NDBL, P, DW), dtype=np.float16)
        for c in range(NDBL):
            k0 = NSING + 2 * c
            k1 = k0 + 1
            xwd[c, :, D_XOFF : D_XOFF + B] = xT[k0 * P : (k0 + 1) * P]
            xwd[c, :, D_XOFF + B : D_XOFF + 2 * B] = xT[k1 * P : (k1 + 1) * P]
            xwd[c, :, D_WOFF : D_WOFF + OUT_L] = wTi[k0 * P : (k0 + 1) * P]
            xwd[c, :, D_WOFF + OUT_L : D_WOFF + 2 * OUT_L] = wTi[
                k1 * P : (k1 + 1) * P
            ]
            xwd[c, :, D_AOFF : D_AOFF + NUM_CLASS] = aT[k0 * P : (k0 + 1) * P]
            xwd[c, :, D_AOFF + NUM_CLASS : DW] = aT[k1 * P : (k1 + 1) * P]

        ppi = pp_base.copy()
        ppi[:NUM_CLASS, BSOFF : BSOFF + OUT_L] = (
            16.0 * lora_B[o0 : o0 + OUT_L, :NUM_CLASS].T
        )
        ppi[NUM_CLASS, BSOFF : BSOFF + OUT_L] = 2.0 * bias[o0 : o0 + OUT_L]
        in_maps.append({"xw_s": xws, "xw_d": xwd, "pp": ppi})
    return in_maps


def kernel(x, pseudo_index, weight, bias, lora_A, lora_B):
    global last_results
    x = np.ascontiguousarray(np.asarray(x, dtype=np.float32))
    pseudo_index = np.ascontiguousarray(np.asarray(pseudo_index, dtype=np.float32))
    weight = np.asarray(weight, dtype=np.float32)
    bias = np.asarray(bias, dtype=np.float32)
    lora_A = np.asarray(lora_A, dtype=np.float32)
    lora_B = np.asarray(lora_B, dtype=np.float32)

    nc = _build()
    in_maps = _pack_inputs(x, pseudo_index, weight, bias, lora_A, lora_B)
    res = run_bass_kernel_spmd(nc, in_maps, list(range(NCORES)))
    last_results = res
    return np.hstack(
        [res.results[i]["out"].astype(np.float32) for i in range(NCORES)]
    )


# revision 38
# speedup vs baseline: 1.1261x; 1.0075x over previous
"""CALoraLinear kernel for 8 TRN2 NeuronCores (Bass/Tile, SPMD).

Math (derived from the reference):
  orig = x @ W.T + bias
  top2 classes c1,c2 per row from pseudo_index[b, :64]
  g_j = <lora_A[c_j], x[b]>          (only rows 0..63 of lora_A are reachable)
  lora_out[b,o] = 16 * sum_c mask[b,c] * G[b,c] * lora_B[o,c]
  out = orig + lora_out + bias       (bias added twice)

Sharding: column-shard W across the 8 cores (each core owns 512 output
columns, full batch); x / lora_A / pseudo_index replicated. Host
concatenates the per-core [512, 512] blocks along the output axis.
(An 8-core G k-split with a DRAM AllReduce was tried and rejected: the
collective measures ~18us internally and starts tens of us late in this
runtime, and enabling collectives adds a global barrier to the preamble.)

Schedule: fp16 operand stream (PE upconverts to FP22; ~3e-4 rel err,
half the DMA bytes of f32r). Four single-k-tile chunks lead (fast-
landing, so the PE builds an SBUF backlog that rides out DMA jitter
without gaps — a PE gap drops the clock p-state and costs ~2.5us),
then 14 double-k-tile chunks. Chunk DMAs alternate the two HWDGE rings
with a window-2 gate per ring: each trigger waits for the
before-previous transfer on its ring, since with more in flight the
DMA queues round-robin across every outstanding transfer and the
first chunk's completion slips by many us. pp (pseudo_index + lora_B
block) rides the GPSIMD SWDGE ring mid-stream. Six dummy matmuls on a
zeroed tile warm the PE clock out of its cold p-state during the
first-chunk DMA wait (measured: steady-state is 216ns per
128x128x512 matmul, cold is 427-610ns, and full clock needs ~3us of
continuous PE activity).

The per-k-tile G matmul uses a 128-col stationary window ending at the
a-block (64 garbage w-cols + the 64 real a-cols, real G in PSUM rows
64:128): a 64-wide stationary forces a PE pipeline-drain costing
~110ns on the G matmul AND the following main (measured 333/322 vs
216ns). The top-2 threshold/mask is computed entirely in the
[class-partition, batch-free] orientation with two GPSIMD
partition_all_reduce maxes (a DRAM-bounce transpose on the SWDGE path
starves behind the HWDGE chunk stream and landed ~15us late). The
last chunk runs both G matmuls first, then closes the four PSUM banks
early->late (mains + rank-128 LoRA tail per bank) so each bank's f16
copy-out (vector/scalar alternating, host upcasts) and store DMA
(triggers on three engines) pipeline against the remaining matmuls;
ht/bS are padded to a 128-row contraction to keep tail geometry
uniform, and ht is produced in per-bank column blocks so each tail
starts as soon as its block is ready.

Evaluated and rejected:
- fp8 e4m3 mains: measures 3.4e-2 full-output rel err, over the 2e-2
  gate (DoubleRow's 2x PE rate can't pay for the 3-matmul error
  compensation it would need).
- G k-split across cores + DRAM AllReduce (would remove 28 of 32
  replicated G matmuls): the collective measures ~18us internally and
  starts tens of us late in this runtime, and enabling collectives
  adds a multi-us global barrier to the preamble.
"""

import os
import sys

for _p in ("/opt/trn_rl_repo",):
    if _p not in sys.path:
        sys.path.insert(0, _p)

import numpy as np

import concourse.bass as bass
import concourse.bass_isa as bass_isa
import concourse.bacc as bacc
import concourse.mybir as mybir
from concourse.tile import TileContext, add_dep_helper
from concourse.bass_utils import run_bass_kernel_spmd


def _ensure_ntff_hook_module():
    """run_bass_kernel_spmd(trace=True) imports antenv.axon_hooks, which the
    agent image's antenv package lacks. Provide it (and register the real
    ctypes NTFF hook when available) so a tracing caller doesn't crash."""
    import types

    try:
        import antenv
    except ImportError:
        return
    if getattr(antenv, "axon_hooks", None) is not None:
        return
    mod = types.ModuleType("antenv.axon_hooks")
    state = {"hook": None}
    mod.set_axon_ntff_profile_hook = lambda h: state.__setitem__("hook", h)
    mod.get_axon_ntff_profile_hook = lambda: state["hook"]
    sys.modules["antenv.axon_hooks"] = mod
    antenv.axon_hooks = mod
    try:
        from trn_agent_boot.trn_boot import _ntff_profile_via_ctypes

        mod.set_axon_ntff_profile_hook(
            _ntff_profile_via_ctypes("/opt/axon/libaxon_pjrt.so")
        )
    except Exception:
        pass


_ensure_ntff_hook_module()

B, IN, OUT = 512, 4096, 4096
NUM_CLASS, RANK = 64, 8
NCORES = 8
OUT_L = OUT // NCORES  # 512
P = 128
KT = IN // P           # 32 k-tiles
BT = B // P            # 4 batch tiles

NSING = 6                  # leading single-k-tile chunks (PE-side cushion:
                           # small chunks land fast, so the stream builds an
                           # SBUF backlog that rides out DMA jitter without
                           # PE gaps, which would drop the clock p-state)
NDBL = (KT - NSING) // 2   # 13 double-k-tile chunks

# single chunk columns: [x: B][w: OUT_L][a: 64]
S_XOFF, S_WOFF, S_AOFF = 0, B, B + OUT_L
SW = B + OUT_L + NUM_CLASS                 # 1088
# double chunk columns: [x0][x1][w0][w1][a0][a1]
D_XOFF, D_WOFF, D_AOFF = 0, 2 * B, 2 * (B + OUT_L)
DW = 2 * SW                                # 2176

# pp layout: [psT: B][bS: OUT_L (rows 0:65, rest zero)]
PTOFF = 0
BSOFF = PTOFF + B
PPW = BSOFF + OUT_L

F32 = mybir.dt.float32
F32R = mybir.dt.float32r
F16 = mybir.dt.float16
X = mybir.AxisListType.X

NWARM = int(os.environ.get("NWARM", "6"))

_cache = {}
# test.py reads this after a traced run for HW exec time
last_results = None


def _build():
    key = f"nc_w{NWARM}"
    if key in _cache:
        return _cache[key]
    nc = bacc.Bacc(
        bass.get_trn_type() or "TRN2",
        target_bir_lowering=False,
        debug=False,
        num_devices=NCORES,
    )

    xw_s = nc.dram_tensor("xw_s", [NSING, P, SW], F16, kind="ExternalInput")
    xw_d = nc.dram_tensor("xw_d", [NDBL, P, DW], F16, kind="ExternalInput")
    pp = nc.dram_tensor("pp", [P, PPW], F32R, kind="ExternalInput")
    # f16 output staging: halves the PSUM->SBUF copy and store-DMA bytes on
    # the critical tail; the host upcasts. Adds ~2^-12 RMS rounding on top
    # of the fp16 stream's ~3e-4 rel err (measured total 3.9e-4).
    out = nc.dram_tensor("out", [B, OUT_L], F16, kind="ExternalOutput")

    with TileContext(nc) as tc:
        with (
            tc.tile_pool(name="xwp", bufs=1) as xwpool,
            tc.tile_pool(name="sml", bufs=1) as spool,
            tc.tile_pool(name="tl", bufs=1) as tpool,
            tc.tile_pool(name="op", bufs=1) as opool,
            tc.tile_pool(name="dr", bufs=1, space="DRAM") as dpool,
            tc.tile_pool(name="ps", bufs=1, space="PSUM") as ppool,
        ):
            # ---- PE warmup: dummy matmuls ramp the PE clock out of its
            # cold p-state while the first chunk DMA is in flight. The
            # result bank is never read.
            if NWARM:
                wt = spool.tile([P, P + OUT_L], F16, tag="warm")
                nc.vector.memset(wt, 0.0)
                warm_ps = ppool.tile([P, OUT_L], F32, tag="warm", name="warm")
                for _ in range(NWARM):
                    nc.tensor.matmul(
                        warm_ps,
                        lhsT=wt[:, :P],
                        rhs=wt[:, P : P + OUT_L],
                        start=True,
                        stop=True,
                    )

            # ---- input DMA triggers on the two HWDGE rings, window-2 gated:
            # each ring holds at most 2 in-flight transfers. With more, the
            # DMA queues round-robin across every outstanding transfer and
            # the FIRST chunk's completion slips by many us (measured: first
            # matmul at 18.6us instead of ~10us when all 17 were issued
            # up-front). pp is sequenced mid-stream on the scalar ring: its
            # consumers (top-2 mask, bS) aren't needed until stream end.
            s_tiles = [
                xwpool.tile([P, SW], F16, tag=f"s{c}", name=f"s{c}")
                for c in range(NSING)
            ]
            d_tiles = [
                xwpool.tile([P, DW], F16, tag=f"d{c}", name=f"d{c}")
                for c in range(NDBL)
            ]
            pp_sb = spool.tile([P, PPW], F32R)
            prev_dma = {0: None, 1: None}  # per-ring (c-2) gating chain
            gate_dma = {0: None, 1: None}

            def issue(eng_i, out_tile, src):
                eng = nc.sync if eng_i == 0 else nc.scalar
                dma = eng.dma_start(out=out_tile, in_=src)
                if gate_dma[eng_i] is not None:
                    add_dep_helper(
                        dma.ins,
                        gate_dma[eng_i].ins,
                        reason="window-2 DMA gating per ring",
                    )
                gate_dma[eng_i] = prev_dma[eng_i]
                prev_dma[eng_i] = dma
                return dma

            s_dmas = [issue(s % 2, s_tiles[s], xw_s[s]) for s in range(NSING)]
            # hand the first doubles an early gate (first single's completion
            # instead of the window-2 chain) so they start ~1.5us sooner —
            # the singles->doubles handoff is where DMA jitter gaps the PE
            for ring in (0, 1):
                gate_dma[ring] = s_dmas[ring]
                prev_dma[ring] = s_dmas[NSING - 2 + ring]
            d_dmas = []
            for c in range(NDBL):
                d_dmas.append(issue(c % 2, d_tiles[c], xw_d[c]))
            # pp rides the GPSIMD SWDGE ring so it never crowds the chunk
            # stream; gated until d1 lands (its consumers run mid-stream)
            pp_dma = nc.gpsimd.dma_start(out=pp_sb, in_=pp[:, :])
            add_dep_helper(
                pp_dma.ins, d_dmas[1].ins, reason="keep pp off the early chunks"
            )

            psT_sb = pp_sb[:NUM_CLASS, PTOFF : PTOFF + B].bitcast(F32)
            bS_sb = pp_sb[:, BSOFF : BSOFF + OUT_L]  # rows 65:128 are zeros

            # ---- PSUM accumulators ----
            mps = [
                ppool.tile([P, OUT_L], F32, tag=f"main{bt}", name=f"main{bt}")
                for bt in range(BT)
            ]
            # G accumulator: full 128-partition bank; the real G lives in
            # rows 64:128. The G matmul's stationary is a 128-col window
            # ending at the a-block (64 w-cols of garbage + the 64 a-cols),
            # so its geometry matches the main matmuls exactly — a 64-wide
            # stationary forces a PE pipeline drain (~110ns) on the G matmul
            # AND on the following main (measured 333/322 vs 216ns).
            gt_ps = ppool.tile([P, B], F32, tag="gt", name="gt_ps")

            def do_k(xk, wk, a128, kidx):
                # G first: at the last k-tile this lets the DVE ht chain
                # overlap the final main matmuls
                nc.tensor.matmul(
                    gt_ps,
                    lhsT=a128,
                    rhs=xk,
                    start=(kidx == 0),
                    stop=(kidx == KT - 1),
                )
                for bt in range(BT):
                    nc.tensor.matmul(
                        mps[bt],
                        lhsT=xk[:, bt * P : (bt + 1) * P],
                        rhs=wk,
                        start=(kidx == 0),
                        stop=False,
                    )

            kidx = 0
            for s in range(NSING):
                t = s_tiles[s]
                do_k(
                    t[:, S_XOFF : S_XOFF + B],
                    t[:, S_WOFF : S_WOFF + OUT_L],
                    t[:, S_AOFF - NUM_CLASS : S_AOFF + NUM_CLASS],
                    kidx,
                )
                kidx += 1

            # ---- top-2 threshold + mask, computed directly in the
            # [class-partition, batch-free] orientation with GPSIMD
            # cross-partition all-reduces. (The previous partition->free
            # DRAM bounce rode the SWDGE path, which starves while the
            # HWDGE chunk stream runs — measured landing ~15us late and
            # stalling the tail matmuls.)
            psT2 = tpool.tile([NUM_CLASS, B], F32)
            nc.vector.tensor_copy(out=psT2, in_=psT_sb)
            m1a = tpool.tile([NUM_CLASS, B], F32)
            nc.gpsimd.partition_all_reduce(
                m1a, psT2, NUM_CLASS, bass_isa.ReduceOp.max
            )
            # kill the per-column max, then the next all-reduce max is the
            # 2nd max = top-2 threshold (replicated across partitions)
            negm = tpool.tile([NUM_CLASS, B], F32)
            nc.vector.tensor_tensor(
                out=negm, in0=psT2, in1=m1a, op=mybir.AluOpType.is_ge
            )
            nc.vector.tensor_scalar(
                out=negm,
                in0=negm,
                scalar1=-1.0e30,
                scalar2=None,
                op0=mybir.AluOpType.mult,
            )
            nc.vector.tensor_tensor(
                out=negm, in0=psT2, in1=negm, op=mybir.AluOpType.add
            )
            thr = tpool.tile([NUM_CLASS, B], F32)
            nc.gpsimd.partition_all_reduce(
                thr, negm, NUM_CLASS, bass_isa.ReduceOp.max
            )
            maskT = tpool.tile([NUM_CLASS, B], F32)
            nc.vector.tensor_tensor(
                out=maskT, in0=psT2, in1=thr, op=mybir.AluOpType.is_ge
            )
            # ht padded to a full 128-row contraction (rows 65:128 zero, with
            # matching zero rows in bS) so the tail matmul keeps the same
            # 128-partition geometry as the stream and avoids the PE drain.
            ht = tpool.tile([P, B], F32R)
            # zero rows 64:128 (aligned base), then the ones row at 64
            # (Memset can't write float32r, hence the in0*0+c idiom)
            nc.vector.tensor_scalar(
                out=ht[NUM_CLASS:P, :],
                in0=psT2[0 : P - NUM_CLASS, :],
                scalar1=0.0,
                scalar2=0.0,
                op0=mybir.AluOpType.mult,
                op1=mybir.AluOpType.add,
            )
            nc.vector.tensor_scalar(
                out=ht[NUM_CLASS : NUM_CLASS + 1, :],
                in0=psT2[0:1, :],
                scalar1=0.0,
                scalar2=1.0,
                op0=mybir.AluOpType.mult,
                op1=mybir.AluOpType.add,
            )

            # ---- double chunks ----
            def d_parts(c, kk):
                t = d_tiles[c]
                return (
                    t[:, D_XOFF + kk * B : D_XOFF + (kk + 1) * B],
                    t[:, D_WOFF + kk * OUT_L : D_WOFF + (kk + 1) * OUT_L],
                    t[
                        :,
                        D_AOFF
                        + (kk - 1) * NUM_CLASS : D_AOFF
                        + (kk + 1) * NUM_CLASS,
                    ],
                )

            for c in range(NDBL - 1):
                for kk in range(2):
                    xk, wk, a128 = d_parts(c, kk)
                    do_k(xk, wk, a128, kidx)
                    kidx += 1

            # last chunk: both G matmuls first so the ht chain overlaps the
            # final mains; then banks close EARLY->LATE (mains kk0/kk1 + the
            # rank-128 LoRA tail per bank, no two consecutive instructions on
            # one bank), so each bank's copy-out + store DMA pipeline against
            # the remaining matmuls.
            last = NDBL - 1
            for kk in range(2):
                xk, _, a128 = d_parts(last, kk)
                nc.tensor.matmul(
                    gt_ps,
                    lhsT=a128,
                    rhs=xk,
                    start=False,
                    stop=(kk == 1),
                )
            kidx += 2

            # ht = G * mask in per-bank column blocks (subtile deps let each
            # tail matmul start as soon as its block is ready), reading the G
            # accumulator PSUM directly (real G in rows 64:128)
            for bt in range(BT):
                nc.vector.tensor_tensor(
                    out=ht[0:NUM_CLASS, bt * P : (bt + 1) * P],
                    in0=gt_ps[NUM_CLASS : 2 * NUM_CLASS, bt * P : (bt + 1) * P],
                    in1=maskT[:, bt * P : (bt + 1) * P],
                    op=mybir.AluOpType.mult,
                )

            o_all = opool.tile([P, BT * OUT_L], F16)

            def close_bank(bt):
                nc.tensor.matmul(
                    mps[bt],
                    lhsT=ht[:, bt * P : (bt + 1) * P],
                    rhs=bS_sb,
                    start=False,
                    stop=True,
                )
                dst = o_all[:, bt * OUT_L : (bt + 1) * OUT_L]
                # copies alternate vector/scalar; store triggers spread
                # across three engines, so per-bank copy+issue overlap
                if bt % 2 == 0:
                    nc.vector.tensor_copy(out=dst, in_=mps[bt])
                else:
                    nc.scalar.copy(out=dst, in_=mps[bt])
                eng = (nc.sync, nc.gpsimd, nc.sync, nc.scalar)[bt]
                eng.dma_start(out=out[bt * P : (bt + 1) * P, :], in_=dst)

            def m_last(kk, bt):
                xk, wk, _ = d_parts(last, kk)
                nc.tensor.matmul(
                    mps[bt],
                    lhsT=xk[:, bt * P : (bt + 1) * P],
                    rhs=wk,
                    start=False,
                    stop=False,
                )

            for b0 in (0, 2):
                m_last(0, b0)
                m_last(0, b0 + 1)
                m_last(1, b0)
                m_last(1, b0 + 1)
                close_bank(b0)
                close_bank(b0 + 1)

    nc.finalize()
    _cache[key] = nc
    return nc


def _pack_inputs(x, pseudo_index, weight, bias, lora_A, lora_B):
    """Build the per-core chunked xw buffers + replicated small inputs."""
    xT = np.ascontiguousarray(x.T).astype(np.float16)        # [IN, B]
    aT = np.ascontiguousarray(
        lora_A[:NUM_CLASS].T
    ).astype(np.float16)                                     # [IN, 64]

    pp_base = np.zeros((P, PPW), dtype=np.float32)
    pp_base[:NUM_CLASS, PTOFF : PTOFF + B] = pseudo_index.T

    in_maps = []
    for i in range(NCORES):
        o0 = i * OUT_L
        wTi = weight[o0 : o0 + OUT_L].T.astype(np.float16)   # [IN, OUT_L]

        xws = np.empty((NSING, P, SW), dtype=np.float16)
        for s in range(NSING):
            xws[s, :, S_XOFF : S_XOFF + B] = xT[s * P : (s + 1) * P]
            xws[s, :, S_WOFF : S_WOFF + OUT_L] = wTi[s * P : (s + 1) * P]
            xws[s, :, S_AOFF : S_AOFF + NUM_CLASS] = aT[s * P : (s + 1) * P]

        xwd = np.empty((NDBL, P, DW), dtype=np.float16)
        for c in range(NDBL):
            k0 = NSING + 2 * c
            k1 = k0 + 1
            xwd[c, :, D_XOFF : D_XOFF + B] = xT[k0 * P : (k0 + 1) * P]
            xwd[c, :, D_XOFF + B : D_XOFF + 2 * B] = xT[k1 * P : (k1 + 1) * P]
            xwd[c, :, D_WOFF : D_WOFF + OUT_L] = wTi[k0 * P : (k0 + 1) * P]
            xwd[c, :, D_WOFF + OUT_L : D_WOFF + 2 * OUT_L] = wTi[
                k1 * P : (k1 + 1) * P
            ]
            xwd[c, :, D_AOFF : D_AOFF + NUM_CLASS] = aT[k0 * P : (k0 + 1) * P]
            xwd[c, :, D_AOFF + NUM_CLASS : DW] = aT[k1 * P : (k1 + 1) * P]

        ppi = pp_base.copy()
        ppi[:NUM_CLASS, BSOFF : BSOFF + OUT_L] = (
            16.0 * lora_B[o0 : o0 + OUT_L, :NUM_CLASS].T
        )
        ppi[NUM_CLASS, BSOFF : BSOFF + OUT_L] = 2.0 * bias[o0 : o0 + OUT_L]
        in_maps.append({"xw_s": xws, "xw_d": xwd, "pp": ppi})
    return in_maps


def kernel(x, pseudo_index, weight, bias, lora_A, lora_B):
    global last_results
    x = np.ascontiguousarray(np.asarray(x, dtype=np.float32))
    pseudo_index = np.ascontiguousarray(np.asarray(pseudo_index, dtype=np.float32))
    weight = np.asarray(weight, dtype=np.float32)
    bias = np.asarray(bias, dtype=np.float32)
    lora_A = np.asarray(lora_A, dtype=np.float32)
    lora_B = np.asarray(lora_B, dtype=np.float32)

    nc = _build()
    in_maps = _pack_inputs(x, pseudo_index, weight, bias, lora_A, lora_B)
    res = run_bass_kernel_spmd(nc, in_maps, list(range(NCORES)))
    last_results = res
    return np.hstack(
        [res.results[i]["out"].astype(np.float32) for i in range(NCORES)]
    )


# revision 39
# speedup vs baseline: 1.2135x; 1.0776x over previous
"""CALoraLinear kernel for 8 TRN2 NeuronCores (Bass/Tile, SPMD).

Math (derived from the reference):
  orig = x @ W.T + bias
  top2 classes c1,c2 per row from pseudo_index[b, :64]
  g_j = <lora_A[c_j], x[b]>          (only rows 0..63 of lora_A are reachable)
  lora_out[b,o] = 16 * sum_c mask[b,c] * G[b,c] * lora_B[o,c]
  out = orig + lora_out + bias       (bias added twice)

Sharding: column-shard W across the 8 cores (each core owns 512 output
columns, full batch); x / lora_A / pseudo_index replicated. Host
concatenates the per-core [512, 512] blocks along the output axis.
(An 8-core G k-split with a DRAM AllReduce was tried and rejected: the
collective measures ~18us internally and starts tens of us late in this
runtime, and enabling collectives adds a global barrier to the preamble.)

Schedule: fp16 operand stream (PE upconverts to FP22; ~3e-4 rel err,
half the DMA bytes of f32r). Four single-k-tile chunks lead (fast-
landing, so the PE builds an SBUF backlog that rides out DMA jitter
without gaps — a PE gap drops the clock p-state and costs ~2.5us),
then 14 double-k-tile chunks. Chunk DMAs alternate the two HWDGE rings
with a window-2 gate per ring: each trigger waits for the
before-previous transfer on its ring, since with more in flight the
DMA queues round-robin across every outstanding transfer and the
first chunk's completion slips by many us. pp (pseudo_index + lora_B
block) rides the GPSIMD SWDGE ring mid-stream. Six dummy matmuls on a
zeroed tile warm the PE clock out of its cold p-state during the
first-chunk DMA wait (measured: steady-state is 216ns per
128x128x512 matmul, cold is 427-610ns, and full clock needs ~3us of
continuous PE activity).

The per-k-tile G matmul uses a 128-col stationary window ending at the
a-block (64 garbage w-cols + the 64 real a-cols, real G in PSUM rows
64:128): a 64-wide stationary forces a PE pipeline-drain costing
~110ns on the G matmul AND the following main (measured 333/322 vs
216ns). The top-2 threshold/mask is computed entirely in the
[class-partition, batch-free] orientation with two GPSIMD
partition_all_reduce maxes (a DRAM-bounce transpose on the SWDGE path
starves behind the HWDGE chunk stream and landed ~15us late). The
last chunk runs both G matmuls first, then closes the four PSUM banks
early->late (mains + rank-128 LoRA tail per bank) so each bank's f16
copy-out (vector/scalar alternating, host upcasts) and store DMA
(triggers on three engines) pipeline against the remaining matmuls;
ht/bS are padded to a 128-row contraction to keep tail geometry
uniform, and ht is produced in per-bank column blocks so each tail
starts as soon as its block is ready.

Evaluated and rejected:
- fp8 e4m3 mains: measures 3.4e-2 full-output rel err, over the 2e-2
  gate (DoubleRow's 2x PE rate can't pay for the 3-matmul error
  compensation it would need).
- G k-split across cores + DRAM AllReduce (would remove 28 of 32
  replicated G matmuls): the collective measures ~18us internally and
  starts tens of us late in this runtime, and enabling collectives
  adds a multi-us global barrier to the preamble.
"""

import os
import sys

for _p in ("/opt/trn_rl_repo",):
    if _p not in sys.path:
        sys.path.insert(0, _p)

import numpy as np

import concourse.bass as bass
import concourse.bass_isa as bass_isa
import concourse.bacc as bacc
import concourse.mybir as mybir
from concourse.tile import TileContext, add_dep_helper
from concourse.bass_utils import run_bass_kernel_spmd


def _ensure_ntff_hook_module():
    """run_bass_kernel_spmd(trace=True) imports antenv.axon_hooks, which the
    agent image's antenv package lacks. Provide it (and register the real
    ctypes NTFF hook when available) so a tracing caller doesn't crash."""
    import types

    try:
        import antenv
    except ImportError:
        return
    if getattr(antenv, "axon_hooks", None) is not None:
        return
    mod = types.ModuleType("antenv.axon_hooks")
    state = {"hook": None}
    mod.set_axon_ntff_profile_hook = lambda h: state.__setitem__("hook", h)
    mod.get_axon_ntff_profile_hook = lambda: state["hook"]
    sys.modules["antenv.axon_hooks"] = mod
    antenv.axon_hooks = mod
    try:
        from trn_agent_boot.trn_boot import _ntff_profile_via_ctypes

        mod.set_axon_ntff_profile_hook(
            _ntff_profile_via_ctypes("/opt/axon/libaxon_pjrt.so")
        )
    except Exception:
        pass


_ensure_ntff_hook_module()

B, IN, OUT = 512, 4096, 4096
NUM_CLASS, RANK = 64, 8
NCORES = 8
OUT_L = OUT // NCORES  # 512
P = 128
KT = IN // P           # 32 k-tiles
BT = B // P            # 4 batch tiles

NSING = 6                  # leading single-k-tile chunks (PE-side cushion:
                           # small chunks land fast, so the stream builds an
                           # SBUF backlog that rides out DMA jitter without
                           # PE gaps, which would drop the clock p-state)
NDBL = (KT - NSING) // 2   # 13 double-k-tile chunks

# single chunk columns: [x: B][w: OUT_L][a: 64]
S_XOFF, S_WOFF, S_AOFF = 0, B, B + OUT_L
SW = B + OUT_L + NUM_CLASS                 # 1088
# double chunk columns: [x0][x1][w0][w1][a0][a1]
D_XOFF, D_WOFF, D_AOFF = 0, 2 * B, 2 * (B + OUT_L)
DW = 2 * SW                                # 2176

# pp layout: [psT: B][bS: OUT_L (rows 0:65, rest zero)]
PTOFF = 0
BSOFF = PTOFF + B
PPW = BSOFF + OUT_L

F32 = mybir.dt.float32
F32R = mybir.dt.float32r
F16 = mybir.dt.float16
X = mybir.AxisListType.X

NWARM = int(os.environ.get("NWARM", "6"))

_cache = {}
# test.py reads this after a traced run for HW exec time
last_results = None


def _build():
    key = f"nc_w{NWARM}"
    if key in _cache:
        return _cache[key]
    nc = bacc.Bacc(
        bass.get_trn_type() or "TRN2",
        target_bir_lowering=False,
        debug=False,
        num_devices=NCORES,
    )

    xw_s = nc.dram_tensor("xw_s", [NSING, P, SW], F16, kind="ExternalInput")
    xw_d = nc.dram_tensor("xw_d", [NDBL, P, DW], F16, kind="ExternalInput")
    pp = nc.dram_tensor("pp", [P, PPW], F32R, kind="ExternalInput")
    # f16 output staging: halves the PSUM->SBUF copy and store-DMA bytes on
    # the critical tail; the host upcasts. Adds ~2^-12 RMS rounding on top
    # of the fp16 stream's ~3e-4 rel err (measured total 3.9e-4).
    out = nc.dram_tensor("out", [B, OUT_L], F16, kind="ExternalOutput")

    with TileContext(nc) as tc:
        with (
            tc.tile_pool(name="xwp", bufs=1) as xwpool,
            tc.tile_pool(name="sml", bufs=1) as spool,
            tc.tile_pool(name="tl", bufs=1) as tpool,
            tc.tile_pool(name="op", bufs=1) as opool,
            tc.tile_pool(name="dr", bufs=1, space="DRAM") as dpool,
            tc.tile_pool(name="ps", bufs=1, space="PSUM") as ppool,
        ):
            # ---- PE warmup: dummy matmuls ramp the PE clock out of its
            # cold p-state while the first chunk DMA is in flight. The
            # result bank is never read.
            if NWARM:
                wt = spool.tile([P, P + OUT_L], F16, tag="warm")
                nc.vector.memset(wt, 0.0)
                warm_ps = ppool.tile([P, OUT_L], F32, tag="warm", name="warm")
                for _ in range(NWARM):
                    nc.tensor.matmul(
                        warm_ps,
                        lhsT=wt[:, :P],
                        rhs=wt[:, P : P + OUT_L],
                        start=True,
                        stop=True,
                    )

            # ---- input DMA triggers on the two HWDGE rings, window-2 gated:
            # each ring holds at most 2 in-flight transfers. With more, the
            # DMA queues round-robin across every outstanding transfer and
            # the FIRST chunk's completion slips by many us (measured: first
            # matmul at 18.6us instead of ~10us when all 17 were issued
            # up-front). pp is sequenced mid-stream on the scalar ring: its
            # consumers (top-2 mask, bS) aren't needed until stream end.
            s_tiles = [
                xwpool.tile([P, SW], F16, tag=f"s{c}", name=f"s{c}")
                for c in range(NSING)
            ]
            d_tiles = [
                xwpool.tile([P, DW], F16, tag=f"d{c}", name=f"d{c}")
                for c in range(NDBL)
            ]
            pp_sb = spool.tile([P, PPW], F32R)
            prev_dma = {0: None, 1: None}  # per-ring (c-2) gating chain
            gate_dma = {0: None, 1: None}

            def issue(eng_i, out_tile, src):
                eng = nc.sync if eng_i == 0 else nc.scalar
                dma = eng.dma_start(out=out_tile, in_=src)
                if gate_dma[eng_i] is not None:
                    add_dep_helper(
                        dma.ins,
                        gate_dma[eng_i].ins,
                        reason="window-2 DMA gating per ring",
                    )
                gate_dma[eng_i] = prev_dma[eng_i]
                prev_dma[eng_i] = dma
                return dma

            for s in range(NSING):
                issue(s % 2, s_tiles[s], xw_s[s])
            d_dmas = []
            for c in range(NDBL):
                d_dmas.append(issue(c % 2, d_tiles[c], xw_d[c]))
            # pp rides the GPSIMD SWDGE ring so it never crowds the chunk
            # stream; gated until d1 lands (its consumers run mid-stream)
            pp_dma = nc.gpsimd.dma_start(out=pp_sb, in_=pp[:, :])
            add_dep_helper(
                pp_dma.ins, d_dmas[1].ins, reason="keep pp off the early chunks"
            )

            psT_sb = pp_sb[:NUM_CLASS, PTOFF : PTOFF + B].bitcast(F32)
            bS_sb = pp_sb[:, BSOFF : BSOFF + OUT_L]  # rows 65:128 are zeros

            # ---- PSUM accumulators ----
            mps = [
                ppool.tile([P, OUT_L], F32, tag=f"main{bt}", name=f"main{bt}")
                for bt in range(BT)
            ]
            # G accumulator: full 128-partition bank; the real G lives in
            # rows 64:128. The G matmul's stationary is a 128-col window
            # ending at the a-block (64 w-cols of garbage + the 64 a-cols),
            # so its geometry matches the main matmuls exactly — a 64-wide
            # stationary forces a PE pipeline drain (~110ns) on the G matmul
            # AND on the following main (measured 333/322 vs 216ns).
            gt_ps = ppool.tile([P, B], F32, tag="gt", name="gt_ps")

            def do_k(xk, wk, a128, kidx):
                # G first: at the last k-tile this lets the DVE ht chain
                # overlap the final main matmuls
                nc.tensor.matmul(
                    gt_ps,
                    lhsT=a128,
                    rhs=xk,
                    start=(kidx == 0),
                    stop=(kidx == KT - 1),
                )
                for bt in range(BT):
                    nc.tensor.matmul(
                        mps[bt],
                        lhsT=xk[:, bt * P : (bt + 1) * P],
                        rhs=wk,
                        start=(kidx == 0),
                        stop=False,
                    )

            kidx = 0
            for s in range(NSING):
                t = s_tiles[s]
                do_k(
                    t[:, S_XOFF : S_XOFF + B],
                    t[:, S_WOFF : S_WOFF + OUT_L],
                    t[:, S_AOFF - NUM_CLASS : S_AOFF + NUM_CLASS],
                    kidx,
                )
                kidx += 1

            # ---- top-2 threshold + mask, computed directly in the
            # [class-partition, batch-free] orientation with GPSIMD
            # cross-partition all-reduces. (The previous partition->free
            # DRAM bounce rode the SWDGE path, which starves while the
            # HWDGE chunk stream runs — measured landing ~15us late and
            # stalling the tail matmuls.)
            psT2 = tpool.tile([NUM_CLASS, B], F32)
            nc.vector.tensor_copy(out=psT2, in_=psT_sb)
            m1a = tpool.tile([NUM_CLASS, B], F32)
            nc.gpsimd.partition_all_reduce(
                m1a, psT2, NUM_CLASS, bass_isa.ReduceOp.max
            )
            # kill the per-column max, then the next all-reduce max is the
            # 2nd max = top-2 threshold (replicated across partitions)
            negm = tpool.tile([NUM_CLASS, B], F32)
            nc.vector.tensor_tensor(
                out=negm, in0=psT2, in1=m1a, op=mybir.AluOpType.is_ge
            )
            nc.vector.tensor_scalar(
                out=negm,
                in0=negm,
                scalar1=-1.0e30,
                scalar2=None,
                op0=mybir.AluOpType.mult,
            )
            nc.vector.tensor_tensor(
                out=negm, in0=psT2, in1=negm, op=mybir.AluOpType.add
            )
            thr = tpool.tile([NUM_CLASS, B], F32)
            nc.gpsimd.partition_all_reduce(
                thr, negm, NUM_CLASS, bass_isa.ReduceOp.max
            )
            maskT = tpool.tile([NUM_CLASS, B], F32)
            nc.vector.tensor_tensor(
                out=maskT, in0=psT2, in1=thr, op=mybir.AluOpType.is_ge
            )
            # ht padded to a full 128-row contraction (rows 65:128 zero, with
            # matching zero rows in bS) so the tail matmul keeps the same
            # 128-partition geometry as the stream and avoids the PE drain.
            ht = tpool.tile([P, B], F32R)
            # zero rows 64:128 (aligned base), then the ones row at 64
            # (Memset can't write float32r, hence the in0*0+c idiom)
            nc.vector.tensor_scalar(
                out=ht[NUM_CLASS:P, :],
                in0=psT2[0 : P - NUM_CLASS, :],
                scalar1=0.0,
                scalar2=0.0,
                op0=mybir.AluOpType.mult,
                op1=mybir.AluOpType.add,
            )
            nc.vector.tensor_scalar(
                out=ht[NUM_CLASS : NUM_CLASS + 1, :],
                in0=psT2[0:1, :],
                scalar1=0.0,
                scalar2=1.0,
                op0=mybir.AluOpType.mult,
                op1=mybir.AluOpType.add,
            )

            # ---- double chunks ----
            def d_parts(c, kk):
                t = d_tiles[c]
                return (
                    t[:, D_XOFF + kk * B : D_XOFF + (kk + 1) * B],
                    t[:, D_WOFF + kk * OUT_L : D_WOFF + (kk + 1) * OUT_L],
                    t[
                        :,
                        D_AOFF
                        + (kk - 1) * NUM_CLASS : D_AOFF
                        + (kk + 1) * NUM_CLASS,
                    ],
                )

            for c in range(NDBL - 1):
                for kk in range(2):
                    xk, wk, a128 = d_parts(c, kk)
                    do_k(xk, wk, a128, kidx)
                    kidx += 1

            # last chunk: both G matmuls first so the ht chain overlaps the
            # final mains; then banks close EARLY->LATE (mains kk0/kk1 + the
            # rank-128 LoRA tail per bank, no two consecutive instructions on
            # one bank), so each bank's copy-out + store DMA pipeline against
            # the remaining matmuls.
            last = NDBL - 1
            for kk in range(2):
                xk, _, a128 = d_parts(last, kk)
                nc.tensor.matmul(
                    gt_ps,
                    lhsT=a128,
                    rhs=xk,
                    start=False,
                    stop=(kk == 1),
                )
            kidx += 2

            # ht = G * mask in per-bank column blocks (subtile deps let each
            # tail matmul start as soon as its block is ready), reading the G
            # accumulator PSUM directly (real G in rows 64:128)
            for bt in range(BT):
                nc.vector.tensor_tensor(
                    out=ht[0:NUM_CLASS, bt * P : (bt + 1) * P],
                    in0=gt_ps[NUM_CLASS : 2 * NUM_CLASS, bt * P : (bt + 1) * P],
                    in1=maskT[:, bt * P : (bt + 1) * P],
                    op=mybir.AluOpType.mult,
                )

            o_all = opool.tile([P, BT * OUT_L], F16)

            def close_bank(bt):
                nc.tensor.matmul(
                    mps[bt],
                    lhsT=ht[:, bt * P : (bt + 1) * P],
                    rhs=bS_sb,
                    start=False,
                    stop=True,
                )
                dst = o_all[:, bt * OUT_L : (bt + 1) * OUT_L]
                # copies alternate vector/scalar; store triggers spread
                # across three engines, so per-bank copy+issue overlap
                if bt % 2 == 0:
                    nc.vector.tensor_copy(out=dst, in_=mps[bt])
                else:
                    nc.scalar.copy(out=dst, in_=mps[bt])
                eng = (nc.sync, nc.gpsimd, nc.sync, nc.scalar)[bt]
                eng.dma_start(out=out[bt * P : (bt + 1) * P, :], in_=dst)

            def m_last(kk, bt):
                xk, wk, _ = d_parts(last, kk)
                nc.tensor.matmul(
                    mps[bt],
                    lhsT=xk[:, bt * P : (bt + 1) * P],
                    rhs=wk,
                    start=False,
                    stop=False,
                )

            for b0 in (0, 2):
                m_last(0, b0)
                m_last(0, b0 + 1)
                m_last(1, b0)
                m_last(1, b0 + 1)
                close_bank(b0)
                close_bank(b0 + 1)

    nc.finalize()
    _cache[key] = nc
    return nc


def _pack_inputs(x, pseudo_index, weight, bias, lora_A, lora_B):
    """Build the per-core chunked xw buffers + replicated small inputs."""
    xT = np.ascontiguousarray(x.T).astype(np.float16)        # [IN, B]
    aT = np.ascontiguousarray(
        lora_A[:NUM_CLASS].T
    ).astype(np.float16)                                     # [IN, 64]

    pp_base = np.zeros((P, PPW), dtype=np.float32)
    pp_base[:NUM_CLASS, PTOFF : PTOFF + B] = pseudo_index.T

    in_maps = []
    for i in range(NCORES):
        o0 = i * OUT_L
        wTi = weight[o0 : o0 + OUT_L].T.astype(np.float16)   # [IN, OUT_L]

        xws = np.empty((NSING, P, SW), dtype=np.float16)
        for s in range(NSING):
            xws[s, :, S_XOFF : S_XOFF + B] = xT[s * P : (s + 1) * P]
            xws[s, :, S_WOFF : S_WOFF + OUT_L] = wTi[s * P : (s + 1) * P]
            xws[s, :, S_AOFF : S_AOFF + NUM_CLASS] = aT[s * P : (s + 1) * P]

        xwd = np.empty((NDBL, P, DW), dtype=np.float16)
        for c in range(NDBL):
            k0 = NSING + 2 * c
            k1 = k0 + 1
            xwd[c, :, D_XOFF : D_XOFF + B] = xT[k0 * P : (k0 + 1) * P]
            xwd[c, :, D_XOFF + B : D_XOFF + 2 * B] = xT[k1 * P : (k1 + 1) * P]
            xwd[c, :, D_WOFF : D_WOFF + OUT_L] = wTi[k0 * P : (k0 + 1) * P]
            xwd[c, :, D_WOFF + OUT_L : D_WOFF + 2 * OUT_L] = wTi[
                k1 * P : (k1 + 1) * P
            ]
            xwd[c, :, D_AOFF : D_AOFF + NUM_CLASS] = aT[k0 * P : (k0 + 1) * P]
            xwd[c, :, D_AOFF + NUM_CLASS : DW] = aT[k1 * P : (k1 + 1) * P]

        ppi = pp_base.copy()
        ppi[:NUM_CLASS, BSOFF : BSOFF + OUT_L] = (
            16.0 * lora_B[o0 : o0 + OUT_L, :NUM_CLASS].T
        )
        ppi[NUM_CLASS, BSOFF : BSOFF + OUT_L] = 2.0 * bias[o0 : o0 + OUT_L]
        in_maps.append({"xw_s": xws, "xw_d": xwd, "pp": ppi})
    return in_maps


def kernel(x, pseudo_index, weight, bias, lora_A, lora_B):
    global last_results
    x = np.ascontiguousarray(np.asarray(x, dtype=np.float32))
    pseudo_index = np.ascontiguousarray(np.asarray(pseudo_index, dtype=np.float32))
    weight = np.asarray(weight, dtype=np.float32)
    bias = np.asarray(bias, dtype=np.float32)
    lora_A = np.asarray(lora_A, dtype=np.float32)
    lora_B = np.asarray(lora_B, dtype=np.float32)

    nc = _build()
    in_maps = _pack_inputs(x, pseudo_index, weight, bias, lora_A, lora_B)
    res = run_bass_kernel_spmd(nc, in_maps, list(range(NCORES)))
    last_results = res
    return np.hstack(
        [res.results[i]["out"].astype(np.float32) for i in range(NCORES)]
    )
